# revision 1
# baseline (speedup 1.0000x reference)
"""Trainium2 Bass kernel for nn_LongTermMemory (retrieval_knn).

reference: cos-sim KNN: best[b] = argmax_m cos(context[b], memory[m]);
return memory[best][None] -> [1, B, D].

Strategy (8 NeuronCores): shard memory [65536, 512] on M -> 8192 rows/core.
Per core (all on device):
  - normalize memory rows (ACT square+accum -> sqrt -> recip), convert to
    bf16, DMA-xbar transpose to get d-on-partitions layout,
  - bf16 matmul sim[b_chunk 128, m 512-groups] against transposed normalized
    context (fp32->bf16 screening),
  - vector-engine max (top-8) + max_index per 4096-segment -> candidate
    indices per (b, segment).
Host: exact fp64 re-rank of the ~128 candidates per b (screening in bf16 is
only used to select candidates; final argmax decided at fp64 precision),
then gather rows. This makes the argmax numerically robust.
"""

import numpy as np
import ml_dtypes

import concourse.bacc as bacc
import concourse.tile as tile
from concourse import mybir
from concourse.bass_utils import run_bass_kernel_spmd

B, D, M_TOT = 512, 512, 65536
C = 8                    # cores
M = M_TOT // C           # 8192 rows per core
P = 128
TB = B // P              # 4 b-chunks
TD = D // P              # 4 d-chunks
TM = M // P              # 64 m-tiles
GM = 4                   # m-tiles per matmul group (N=512 moving)
NG = TM // GM            # 16 groups
Q = 2                    # max segments per b-chunk
SEG = M // Q             # 4096
F32 = mybir.dt.float32
BF16 = mybir.dt.bfloat16
U32 = mybir.dt.uint32

_NC_CACHE = {}


def build_nc(skip=()):
    key = ("nc",) + tuple(sorted(skip))
    if key in _NC_CACHE:
        return _NC_CACHE[key]
    from contextlib import ExitStack

    nc = bacc.Bacc("TRN2", target_bir_lowering=False, debug=False)
    ctx_dram = nc.dram_tensor("ctx", [B, D], F32, kind="ExternalInput")
    mem_dram = nc.dram_tensor("mem", [M, D], F32, kind="ExternalInput")
    eye_dram = nc.dram_tensor("eye", [P, P], BF16, kind="ExternalInput")
    cv_dram = nc.dram_tensor("cand_v", [TB, Q, P, 8], BF16, kind="ExternalOutput")
    ci_dram = nc.dram_tensor("cand_i", [TB, Q, P, 8], U32, kind="ExternalOutput")

    with tile.TileContext(nc) as tc, ExitStack() as ex:
        big = ex.enter_context(tc.tile_pool(name="big", bufs=1))
        mp = ex.enter_context(tc.tile_pool(name="mp", bufs=6))
        sq = ex.enter_context(tc.tile_pool(name="sq", bufs=2))
        nb = ex.enter_context(tc.tile_pool(name="nb", bufs=4))
        sm = ex.enter_context(tc.tile_pool(name="sm", bufs=4))
        ps = ex.enter_context(tc.tile_pool(name="ps", bufs=4, space="PSUM"))
        xs = ex.enter_context(tc.tile_pool(name="xs", bufs=3, space="PSUM"))

        # persistent buffers
        ctxT = big.tile([P, TB, TD, P], BF16)        # [d_low, beta, j, b_low]
        memT = big.tile([P, TM, TD, P], BF16)        # [d_low, t, j, m_low]
        simb = big.tile([P, TB, M], BF16)            # [b_low, beta, m]
        ssq = big.tile([P, TM], F32)
        srt = big.tile([P, TM], F32)
        rin = big.tile([P, TM], F32)

        eye = big.tile([P, P], BF16)
        nc.sync.dma_start(eye[:], eye_dram[:])

        # ---- context prep: normalize + bf16 + transpose ----
        for b in range(TB):
            cf = mp.tile([P, D], F32, tag="cf")
            nc.sync.dma_start(cf[:], ctx_dram[b * P:(b + 1) * P, :])
            csq = sq.tile([P, 1], F32, tag="csq")
            cdump = sq.tile([P, D], BF16, tag="cdump")
            nc.scalar.activation(cdump[:], cf[:],
                                 mybir.ActivationFunctionType.Square,
                                 accum_out=csq[:])
            csr = sq.tile([P, 1], F32, tag="csr")
            nc.scalar.sqrt(csr[:], csq[:])
            cri = sq.tile([P, 1], F32, tag="cri")
            nc.vector.reciprocal(cri[:], csr[:])
            cnb = nb.tile([P, D], BF16, tag="cnb")
            nc.vector.tensor_scalar_mul(cnb[:], cf[:], cri[:])
            cxp = xs.tile([P, TD, P], BF16, tag="xp")
            for j in range(TD):
                nc.tensor.transpose(cxp[:, j, :], cnb[:, j * P:(j + 1) * P],
                                    eye[:])
            nc.scalar.copy(ctxT[:, b, :, :], cxp[:])

        # ---- interleaved: per 4-tile group, prep then 4 b-chunk matmuls ----
        for g in range(NG):
            for dt in range(GM):
                t = g * GM + dt
                mf = mp.tile([P, D], F32, tag="mf")
                nc.sync.dma_start(mf[:], mem_dram[t * P:(t + 1) * P, :])
                dump = sq.tile([P, D], BF16, tag="dump")
                nc.scalar.activation(dump[:], mf[:],
                                     mybir.ActivationFunctionType.Square,
                                     accum_out=ssq[:, t:t + 1])
                nc.scalar.sqrt(srt[:, t:t + 1], ssq[:, t:t + 1])
                nc.vector.reciprocal(rin[:, t:t + 1], srt[:, t:t + 1])
                mnb = nb.tile([P, D], BF16, tag="mnb")
                nc.vector.tensor_scalar_mul(mnb[:], mf[:], rin[:, t:t + 1])
                mxp = xs.tile([P, TD, P], BF16, tag="xp")
                for j in range(TD):
                    nc.tensor.transpose(mxp[:, j, :],
                                        mnb[:, j * P:(j + 1) * P], eye[:])
                if t % 2 == 0:
                    nc.vector.tensor_copy(memT[:, t, :, :], mxp[:])
                else:
                    nc.scalar.copy(memT[:, t, :, :], mxp[:])
            for b in range(TB):
                acc = ps.tile([P, GM * P], F32, tag="acc")
                for j in range(TD):
                    nc.tensor.matmul(
                        acc[:],
                        ctxT[:, b, j, :],
                        memT[:, g * GM:(g + 1) * GM, j, :],
                        start=(j == 0), stop=(j == TD - 1),
                    )
                if (b + g) % 2 == 0:
                    nc.scalar.copy(simb[:, b, g * GM * P:(g + 1) * GM * P],
                                   acc[:])
                else:
                    nc.vector.tensor_copy(
                        simb[:, b, g * GM * P:(g + 1) * GM * P], acc[:])
            if g == NG // 2 - 1:
                for b in range(TB):
                    t8v = sm.tile([P, 8], BF16, tag="t8v")
                    t8i = sm.tile([P, 8], U32, tag="t8i")
                    nc.vector.max(t8v[:], simb[:, b, 0:SEG])
                    nc.vector.max_index(t8i[:], t8v[:], simb[:, b, 0:SEG])
                    nc.gpsimd.dma_start(cv_dram[b, 0], t8v[:])
                    nc.gpsimd.dma_start(ci_dram[b, 0], t8i[:])

        # ---- top8 per (b-chunk, segment) ----
        for b in range(TB):
            for q in range(1, Q):
                t8v = sm.tile([P, 8], BF16, tag="t8v")
                t8i = sm.tile([P, 8], U32, tag="t8i")
                nc.vector.max(t8v[:], simb[:, b, q * SEG:(q + 1) * SEG])
                nc.vector.max_index(t8i[:], t8v[:],
                                    simb[:, b, q * SEG:(q + 1) * SEG])
                nc.gpsimd.dma_start(cv_dram[b, q], t8v[:])
                nc.gpsimd.dma_start(ci_dram[b, q], t8i[:])

    nc.compile()
    _NC_CACHE[key] = nc
    return nc


def run_device(context, memory, trace=False):
    nc = build_nc()
    eye = np.eye(P, dtype=ml_dtypes.bfloat16)
    in_maps = [
        {"ctx": np.ascontiguousarray(context),
         "mem": np.ascontiguousarray(memory[c * M:(c + 1) * M]),
         "eye": eye}
        for c in range(C)
    ]
    res = run_bass_kernel_spmd(nc, in_maps, list(range(C)), trace=trace)
    return res


def kernel(context: np.ndarray, memory: np.ndarray) -> np.ndarray:
    res = run_device(context, memory)
    # ---- host: gather candidates, exact fp64 re-rank, gather rows ----
    cand = np.full((B, C * Q * 8), -1, dtype=np.int64)
    for c in range(C):
        ci = res.results[c]["cand_i"].astype(np.int64)  # [TB, Q, P, 8]
        for bt in range(TB):
            for q in range(Q):
                cols = slice((c * Q + q) * 8, (c * Q + q) * 8 + 8)
                cand[bt * P:(bt + 1) * P, cols] = (
                    c * M + q * SEG + ci[bt, q])
    ctx64 = context.astype(np.float64)
    mem64 = memory.astype(np.float64)
    ctxn = ctx64 / np.sqrt(np.maximum((ctx64 * ctx64).sum(1, keepdims=True),
                                      1e-12))
    mnorm = np.sqrt(np.maximum((mem64 * mem64).sum(1), 1e-12))
    # cos[b, k] for candidate k of context b
    rows = mem64[cand]                                  # [B, K, D]
    cos = np.einsum("bd,bkd->bk", ctxn, rows) / mnorm[cand]
    # argmax with smallest-index tie-break
    best = np.empty(B, dtype=np.int64)
    for b in range(B):
        cb, vb = cand[b], cos[b]
        mx = vb.max()
        best[b] = cb[vb >= mx].min()
    return memory[best][None, :, :].astype(np.float32)



# revision 3
# speedup vs baseline: 3.5619x; 3.5619x over previous
"""Trainium2 Bass kernel for nn_LongTermMemory (retrieval_knn).

reference: best[b] = argmax_m cos(context[b], memory[m]); return
memory[best][None] -> [1, B, D].

Strategy (8 NeuronCores, memory sharded on M -> 8192 rows/core):
  Host prep (cheap numpy, all inside kernel()):
    - L2-normalize memory rows and context rows in fp32, scale by 64,
      quantize to fp8 e4m3, transpose to d-major layout per core.
  Device per core (all screening, fp8/fp16):
    - fp8 DoubleRow matmuls: sim[b 128, m 512] f32 in PSUM, K=512 in
      2 instructions (256 contraction each).
    - PSUM pair-drains (2 banks / op) on ACT -> fp16 slabs for most
      (g, b) blocks; DVE direct max-folds from PSUM for the rest.
    - DVE quad tensor_tensor-max folds (fp16, 2x mode) collapse the 16
      m-groups of each b-chunk to one [128, 512] array = max over
      groups at each in-group position.
    - DVE Max/MaxIndex top-8 per b row -> 8 positions per (core, b).
  Host post: candidates = {core*8192 + g*512 + pos} for all 16 g
  (position multiplicity), fp32 cosine re-rank of 1024 candidates/row,
  exact fp64 re-rank of the top 16, smallest-index tie-break, gather.

Screening margin: fp8 dot noise sigma ~9 units (of 4096-scaled sims),
gap between the global max and the 8th-best folded position is ~15
sigma, and the true argmax position is by construction the top-1
folded value of its core, so top-8 position selection cannot lose it
short of astronomically unlikely noise.
"""

import numpy as np
import ml_dtypes

import concourse.bacc as bacc
import concourse.tile as tile
from concourse import mybir
from concourse.bass_utils import run_bass_kernel_spmd

B, D, M_TOT = 512, 512, 65536
C = 8                    # cores
M = M_TOT // C           # 8192 rows per core
P = 128
TB = B // P              # 4 b-chunks
NG = M // 512            # 16 m-groups of 512
QSCALE = 64.0            # pre-quantization scale (exact power of 2)

F32 = mybir.dt.float32
FP16 = mybir.dt.float16
FP8 = mybir.dt.float8e4
U32 = mybir.dt.uint32
DR = mybir.MatmulPerfMode.DoubleRow
MAX = mybir.AluOpType.max

# chunk 3: pairs (g in 8..15) are direct DVE drain-folds; everything else
# (chunks 0..2 fully, chunk 3 g in 0..7) is ACT pair-drained into fp16
# slabs and quad-folded on DVE.
NSLAB = {0: 16, 1: 16, 2: 16, 3: 8}
SLAB_OFF = {0: 0, 1: 16, 2: 32, 3: 48}

_NC_CACHE = {}


def build_nc():
    if "nc" in _NC_CACHE:
        return _NC_CACHE["nc"]
    from contextlib import ExitStack

    nc = bacc.Bacc("TRN2", target_bir_lowering=False, debug=False)
    ctx_d = nc.dram_tensor("ctxT8", [P, 4, B], FP8, kind="ExternalInput")
    mem_d = nc.dram_tensor("memT8", [P, 4, M], FP8, kind="ExternalInput")
    ci_d = nc.dram_tensor("ci", [P, TB, 8], U32, kind="ExternalOutput")

    with tile.TileContext(nc) as tc, ExitStack() as ex:
        big = ex.enter_context(tc.tile_pool(name="big", bufs=1))
        ps = ex.enter_context(tc.tile_pool(name="ps", bufs=1, space="PSUM"))

        ctx8 = big.tile([P, 4, B], FP8)
        mem8 = big.tile([P, 4, M], FP8)
        simb = big.tile([P, 56, 512], FP16)
        runq = big.tile([P, TB, 4, 512], FP16)
        run2 = big.tile([P, 2, 512], F32)       # chunk-3 direct fold
        rq2 = big.tile([P, TB, 2, 512], FP16)
        runb = big.tile([P, TB, 512], FP16)
        rdm = big.tile([P, 512], FP16)          # chunk-3 run2 merge
        t8v = big.tile([P, TB, 8], FP16)
        t8i = big.tile([P, TB, 8], U32)

        acc = [ps.tile([P, 2, 512], F32, tag=f"acc{b}", name=f"acc{b}")
               for b in range(TB)]

        nc.sync.dma_start(ctx8[:], ctx_d[:])
        NCH = 8
        mw = M // NCH
        for k in range(NCH):
            nc.sync.dma_start(mem8[:, :, k * mw:(k + 1) * mw],
                              mem_d[:, :, k * mw:(k + 1) * mw])

        for g in range(NG):
            sl = g % 2
            for b in range(TB):
                a = acc[b][:, sl, :]
                ms = slice(g * 512, (g + 1) * 512)
                bs = slice(b * P, (b + 1) * P)
                nc.tensor.matmul(a, ctx8[:, 0:2, bs], mem8[:, 0:2, ms],
                                 start=True, stop=False, perf_mode=DR)
                nc.tensor.matmul(a, ctx8[:, 2:4, bs], mem8[:, 2:4, ms],
                                 start=False, stop=True, perf_mode=DR)
            if sl == 1:
                pair = g // 2
                for b in range(TB):
                    if b == 3 and g >= 8:
                        if g == 9:
                            nc.vector.tensor_copy(run2[:], acc[b][:])
                        else:
                            nc.vector.tensor_tensor(run2[:], acc[b][:],
                                                    run2[:], MAX)
                    else:
                        s = SLAB_OFF[b] + 2 * pair
                        nc.scalar.copy(simb[:, s:s + 2, :], acc[b][:])
            # quad folds for drained slabs as they complete
            if g in (7, 15):
                half = 0 if g == 7 else 1
                for b in range(TB):
                    if b == 3 and half == 1:
                        continue
                    for q in (2 * half, 2 * half + 1):
                        quad = simb[:, SLAB_OFF[b] + 4 * q:SLAB_OFF[b] + 4 * q + 4, :]
                        if q == 0:
                            nc.vector.tensor_copy(runq[:, b, :, :], quad)
                        else:
                            nc.vector.tensor_tensor(runq[:, b, :, :], quad,
                                                    runq[:, b, :, :], MAX)

        for b in range(TB):
            if b < 3:
                nc.vector.tensor_tensor(rq2[:, b, :, :], runq[:, b, 0:2, :],
                                        runq[:, b, 2:4, :], MAX)
                nc.vector.tensor_tensor(runb[:, b, :], rq2[:, b, 0, :],
                                        rq2[:, b, 1, :], MAX)
            else:
                # chunk 3: 8 drained slabs live in runq[:, 3, 0:4] (2 quads),
                # direct-fold result in run2
                nc.vector.tensor_tensor(rq2[:, b, :, :], runq[:, b, 0:2, :],
                                        runq[:, b, 2:4, :], MAX)
                nc.vector.tensor_tensor(rdm[:], run2[:, 0, :],
                                        run2[:, 1, :], MAX)
                nc.vector.tensor_tensor(runb[:, b, :], rq2[:, b, 0, :],
                                        rq2[:, b, 1, :], MAX)
                nc.vector.tensor_tensor(runb[:, b, :], rdm[:],
                                        runb[:, b, :], MAX)
            nc.vector.max(t8v[:, b, :], runb[:, b, :])
            nc.vector.max_index(t8i[:, b, :], t8v[:, b, :], runb[:, b, :])

        nc.sync.dma_start(ci_d[:], t8i[:])

    nc.compile()
    _NC_CACHE["nc"] = nc
    return nc


def _host_prep(context, memory):
    ctx = np.ascontiguousarray(context, dtype=np.float32)
    mem = np.ascontiguousarray(memory, dtype=np.float32)
    mem_n2 = np.maximum((mem * mem).sum(1, keepdims=True), 1e-12)
    mem_n = mem / np.sqrt(mem_n2)
    ctx_n2 = np.maximum((ctx * ctx).sum(1, keepdims=True), 1e-12)
    ctx_n = ctx / np.sqrt(ctx_n2)

    ctx8 = (ctx_n * QSCALE).astype(ml_dtypes.float8_e4m3)
    mem8 = (mem_n * QSCALE).astype(ml_dtypes.float8_e4m3)

    ctxT8 = np.ascontiguousarray(
        ctx8.T.reshape(4, P, B).transpose(1, 0, 2))
    mem_shards = []
    for c in range(C):
        q = mem8[c * M:(c + 1) * M]
        mem_shards.append(np.ascontiguousarray(
            q.T.reshape(4, P, M).transpose(1, 0, 2)))
    return ctx_n, mem_n, ctxT8, mem_shards


def run_device(context, memory, trace=False):
    nc = build_nc()
    _, _, ctxT8, mem_shards = _host_prep(context, memory)
    in_maps = [{"ctxT8": ctxT8, "memT8": mem_shards[c]} for c in range(C)]
    return run_bass_kernel_spmd(nc, in_maps, list(range(C)), trace=trace)


def kernel(context: np.ndarray, memory: np.ndarray) -> np.ndarray:
    nc = build_nc()
    ctx_n, mem_n, ctxT8, mem_shards = _host_prep(context, memory)
    in_maps = [{"ctxT8": ctxT8, "memT8": mem_shards[c]} for c in range(C)]
    res = run_bass_kernel_spmd(nc, in_maps, list(range(C)))

    # positions: [C, P, TB, 8] -> [b, c, k]
    pos = np.stack([res.results[c]["ci"] for c in range(C)], axis=0)
    pos_b = pos.transpose(2, 1, 0, 3).reshape(B, C, 8).astype(np.int64)
    g = np.arange(NG, dtype=np.int64)
    cand = (np.arange(C, dtype=np.int64)[None, :, None, None] * M
            + g[None, None, None, :] * 512
            + pos_b[:, :, :, None]).reshape(B, C * 8 * NG)

    # fp32 cosine prefilter over the 1024 candidates per row
    KTOP = 16
    best16 = np.empty((B, KTOP), dtype=np.int64)
    for b0 in range(0, B, 64):
        b1 = b0 + 64
        rows = mem_n[cand[b0:b1]]                      # [64, K, D] f32
        sc = np.einsum("bd,bkd->bk", ctx_n[b0:b1], rows)
        part = np.argpartition(-sc, KTOP - 1, axis=1)[:, :KTOP]
        best16[b0:b1] = np.take_along_axis(cand[b0:b1], part, axis=1)

    # exact fp64 re-rank of the survivors, smallest-index tie-break
    ctx64 = context.astype(np.float64)
    mem64 = memory.astype(np.float64)
    ctxn64 = ctx64 / np.sqrt(np.maximum((ctx64 * ctx64).sum(1, keepdims=True),
                                        1e-12))
    mnorm = np.sqrt(np.maximum((mem64 * mem64).sum(1), 1e-12))
    rows64 = mem64[best16]                             # [B, 16, D]
    cos = np.einsum("bd,bkd->bk", ctxn64, rows64) / mnorm[best16]
    best = np.empty(B, dtype=np.int64)
    for b in range(B):
        cb, vb = best16[b], cos[b]
        mx = vb.max()
        best[b] = cb[vb >= mx].min()
    return memory[best][None, :, :].astype(np.float32)


# revision 31
# speedup vs baseline: 4.4652x; 1.2536x over previous
"""Trainium2 Bass kernel for nn_LongTermMemory (retrieval_knn).

reference: best[b] = argmax_m cos(context[b], memory[m]); return
memory[best][None] -> [1, B, D].

Strategy (8 NeuronCores, memory sharded on M -> 8192 rows/core):
  Host prep (cheap numpy, all inside kernel()):
    - L2-normalize memory rows and context rows in fp32, scale by 64,
      quantize to fp8 e4m3, transpose to d-major layout per core.
  Device per core (all screening, fp8/fp16):
    - fp8 DoubleRow matmuls: sim[b 128, m 512] f32 in PSUM, K=512 in
      2 instructions (256 contraction each).
    - PSUM pair-drains (2 banks / op) on ACT -> fp16 slabs for most
      (g, b) blocks; DVE direct max-folds from PSUM for the rest.
    - DVE quad tensor_tensor-max folds (fp16, 2x mode) collapse the 16
      m-groups of each b-chunk to one [128, 512] array = max over
      groups at each in-group position.
    - DVE Max/MaxIndex top-8 per b row -> 8 positions per (core, b).
  Host post: candidates = {core*8192 + g*512 + pos} for all 16 g
  (position multiplicity), fp32 cosine re-rank of 1024 candidates/row,
  exact fp64 re-rank of the top 16, smallest-index tie-break, gather.

Screening margin: fp8 dot noise sigma ~9 units (of 4096-scaled sims),
gap between the global max and the 8th-best folded position is ~15
sigma, and the true argmax position is by construction the top-1
folded value of its core, so top-8 position selection cannot lose it
short of astronomically unlikely noise.
"""

import numpy as np
import ml_dtypes

import concourse.bacc as bacc
import concourse.tile as tile
from concourse import mybir
from concourse.bass_utils import run_bass_kernel_spmd

B, D, M_TOT = 512, 512, 65536
C = 8                    # cores
M = M_TOT // C           # 8192 rows per core
P = 128
TB = B // P              # 4 b-chunks
NG = M // 512            # 16 m-groups of 512
QSCALE = 64.0            # pre-quantization scale (exact power of 2)

F32 = mybir.dt.float32
FP16 = mybir.dt.float16
FP8 = mybir.dt.float8e4
U32 = mybir.dt.uint32
DR = mybir.MatmulPerfMode.DoubleRow
MAX = mybir.AluOpType.max

# chunk 3: pairs >= ACT_PAIRS3 are direct DVE drain-folds; everything else
# is ACT pair-drained into fp16 slabs and pair-folded on DVE.
ACT_PAIRS3 = 3
SLAB_OFF = {0: 0, 1: 16, 2: 32, 3: 48}

_NC_CACHE = {}


def build_nc():
    if "nc" in _NC_CACHE:
        return _NC_CACHE["nc"]
    from contextlib import ExitStack

    nc = bacc.Bacc("TRN2", target_bir_lowering=False, debug=False)
    ctx_d = nc.dram_tensor("ctxT8", [P, 4, B], FP8, kind="ExternalInput")
    mem_d = nc.dram_tensor("memT8", [P, 4, M], FP8, kind="ExternalInput")
    rb_d = nc.dram_tensor("rb", [P, TB, 512], FP16, kind="ExternalOutput")

    with tile.TileContext(nc) as tc, ExitStack() as ex:
        big = ex.enter_context(tc.tile_pool(name="big", bufs=1))
        ps = ex.enter_context(tc.tile_pool(name="ps", bufs=1, space="PSUM"))

        ctx8 = big.tile([P, 4, B], FP8)
        mem8 = big.tile([P, 4, M], FP8)
        simb = big.tile([P, 60, 512], FP16)     # ACT-drained slabs
        run2 = big.tile([P, 2, 512], F32)       # chunk-3 direct fold
        rdm = big.tile([P, 512], FP16)
        runb = big.tile([P, TB, 512], FP16)     # single running max per chunk

        acc = [ps.tile([P, 2, 512], F32, tag=f"acc{b}", name=f"acc{b}")
               for b in range(TB)]

        for b in range(TB):
            bs = slice(b * P, (b + 1) * P)
            nc.sync.dma_start(ctx8[:, :, bs], ctx_d[:, :, bs])
        for k in range(NG):
            nc.gpsimd.dma_start(mem8[:, :, k * 512:(k + 1) * 512],
                                mem_d[:, :, k * 512:(k + 1) * 512])

        # PE warm-up during the DMA lead-in: ~3us of continuous dummy
        # matmuls ramps the PE p-state to full speed before real work
        warm = big.tile([P, P], FP8)
        nc.gpsimd.memset(warm[:], 0)
        for _ in range(25):
            nc.tensor.matmul(acc[0][:, 0, 0:P], warm[:], warm[:],
                             start=True, stop=True)

        def fold_slab(b, s):
            if s == 0:
                nc.vector.tensor_copy(runb[:, b, :], simb[:, SLAB_OFF[b], :])
            else:
                nc.vector.tensor_tensor(runb[:, b, :],
                                        simb[:, SLAB_OFF[b] + s, :],
                                        runb[:, b, :], MAX)

        for g in range(NG):
            sl = g % 2
            border = (3, 2, 0, 1) if g >= 14 else range(TB)
            for b in border:
                a = acc[b][:, sl, :]
                ms = slice(g * 512, (g + 1) * 512)
                bs = slice(b * P, (b + 1) * P)
                nc.tensor.matmul(a, ctx8[:, 0:2, bs], mem8[:, 0:2, ms],
                                 start=True, stop=False, perf_mode=DR)
                nc.tensor.matmul(a, ctx8[:, 2:4, bs], mem8[:, 2:4, ms],
                                 start=False, stop=True, perf_mode=DR)
            if g <= 1:
                # lead-in: ACT single drains for c0/c1 (earliest ACT start),
                # DVE drains c2/c3 straight from PSUM in parallel
                for b in range(TB):
                    if b < 2:
                        nc.scalar.copy(simb[:, SLAB_OFF[b] + g, :],
                                       acc[b][:, sl, :])
                        fold_slab(b, g)
                    elif g == 0:
                        nc.vector.tensor_copy(runb[:, b, :], acc[b][:, sl, :])
                    else:
                        nc.vector.tensor_tensor(runb[:, b, :],
                                                acc[b][:, sl, :],
                                                runb[:, b, :], MAX)
                continue
            if g >= 14:
                # tail: single-slab drains/folds so nothing batches behind
                # column 15; DVE PSUM-direct work first (no ACT dependency)
                nc.vector.tensor_tensor(runb[:, 3, :], acc[3][:, sl, :],
                                        runb[:, 3, :], MAX)
                nc.vector.tensor_tensor(runb[:, 2, :], acc[2][:, sl, :],
                                        runb[:, 2, :], MAX)
                for b in range(2):
                    nc.scalar.copy(simb[:, SLAB_OFF[b] + g, :],
                                   acc[b][:, sl, :])
                    fold_slab(b, g)
                if g == 15:
                    for b in (3, 2, 0, 1):
                        nc.sync.dma_start(rb_d[:, b, :], runb[:, b, :])
                continue
            if sl == 1:
                pair = g // 2
                # chunk 3: direct DVE drain-fold for later pairs
                if pair >= ACT_PAIRS3:
                    if pair == ACT_PAIRS3:
                        nc.vector.tensor_copy(run2[:], acc[3][:])
                    else:
                        nc.vector.tensor_tensor(run2[:], acc[3][:],
                                                run2[:], MAX)
                    if pair == 6:
                        # pre-merge c3's two fold lanes off the tail path
                        nc.vector.tensor_tensor(rdm[:], run2[:, 0, :],
                                                run2[:, 1, :], MAX)
                        nc.vector.tensor_tensor(runb[:, 3, :], rdm[:],
                                                runb[:, 3, :], MAX)
                for b in range(TB):
                    if b == 3 and pair >= ACT_PAIRS3:
                        continue
                    s = SLAB_OFF[b] + 2 * pair
                    nc.scalar.copy(simb[:, s:s + 2, :], acc[b][:])
                    fold_slab(b, 2 * pair)
                    fold_slab(b, 2 * pair + 1)

    nc.compile()
    _NC_CACHE["nc"] = nc
    return nc


def _host_prep(context, memory):
    ctx = np.ascontiguousarray(context, dtype=np.float32)
    mem = np.ascontiguousarray(memory, dtype=np.float32)
    mem_n2 = np.maximum((mem * mem).sum(1, keepdims=True), 1e-12)
    mem_n = mem / np.sqrt(mem_n2)
    ctx_n2 = np.maximum((ctx * ctx).sum(1, keepdims=True), 1e-12)
    ctx_n = ctx / np.sqrt(ctx_n2)

    ctx8 = (ctx_n * QSCALE).astype(ml_dtypes.float8_e4m3)
    mem8 = (mem_n * QSCALE).astype(ml_dtypes.float8_e4m3)

    ctxT8 = np.ascontiguousarray(
        ctx8.T.reshape(4, P, B).transpose(1, 0, 2))
    mem_shards = []
    for c in range(C):
        q = mem8[c * M:(c + 1) * M]
        mem_shards.append(np.ascontiguousarray(
            q.T.reshape(4, P, M).transpose(1, 0, 2)))
    return ctx_n, mem_n, ctxT8, mem_shards


def run_device(context, memory, trace=False):
    nc = build_nc()
    _, _, ctxT8, mem_shards = _host_prep(context, memory)
    in_maps = [{"ctxT8": ctxT8, "memT8": mem_shards[c]} for c in range(C)]
    return run_bass_kernel_spmd(nc, in_maps, list(range(C)), trace=trace)


def kernel(context: np.ndarray, memory: np.ndarray) -> np.ndarray:
    nc = build_nc()
    ctx_n, mem_n, ctxT8, mem_shards = _host_prep(context, memory)
    in_maps = [{"ctxT8": ctxT8, "memT8": mem_shards[c]} for c in range(C)]
    res = run_bass_kernel_spmd(nc, in_maps, list(range(C)))

    # folded max arrays [C, P, TB, 512] -> [b, c, 512]; host top-8 positions
    rb = np.stack([res.results[c]["rb"] for c in range(C)], axis=0)
    rb_b = rb.transpose(2, 1, 0, 3).reshape(B, C, 512).astype(np.float32)
    pos_b = np.argpartition(-rb_b, 7, axis=2)[:, :, :8].astype(np.int64)
    g = np.arange(NG, dtype=np.int64)
    cand = (np.arange(C, dtype=np.int64)[None, :, None, None] * M
            + g[None, None, None, :] * 512
            + pos_b[:, :, :, None]).reshape(B, C * 8 * NG)

    # fp32 cosine prefilter over the 1024 candidates per row
    KTOP = 16
    best16 = np.empty((B, KTOP), dtype=np.int64)
    for b0 in range(0, B, 64):
        b1 = b0 + 64
        rows = mem_n[cand[b0:b1]]                      # [64, K, D] f32
        sc = np.einsum("bd,bkd->bk", ctx_n[b0:b1], rows)
        part = np.argpartition(-sc, KTOP - 1, axis=1)[:, :KTOP]
        best16[b0:b1] = np.take_along_axis(cand[b0:b1], part, axis=1)

    # exact fp64 re-rank of the survivors, smallest-index tie-break
    ctx64 = context.astype(np.float64)
    mem64 = memory.astype(np.float64)
    ctxn64 = ctx64 / np.sqrt(np.maximum((ctx64 * ctx64).sum(1, keepdims=True),
                                        1e-12))
    mnorm = np.sqrt(np.maximum((mem64 * mem64).sum(1), 1e-12))
    rows64 = mem64[best16]                             # [B, 16, D]
    cos = np.einsum("bd,bkd->bk", ctxn64, rows64) / mnorm[best16]
    best = np.empty(B, dtype=np.int64)
    for b in range(B):
        cb, vb = best16[b], cos[b]
        mx = vb.max()
        best[b] = cb[vb >= mx].min()
    return memory[best][None, :, :].astype(np.float32)


# revision 47
# speedup vs baseline: 4.8364x; 1.0831x over previous
"""Trainium2 Bass kernel for nn_LongTermMemory (retrieval_knn).

reference: best[b] = argmax_m cos(context[b], memory[m]); return
memory[best][None] -> [1, B, D].

Strategy (8 NeuronCores, memory sharded on M -> 8192 rows/core):
  Host prep (cheap numpy, all inside kernel()):
    - L2-normalize memory rows and context rows in fp32, scale by 64,
      quantize to fp8 e4m3, transpose to d-major layout per core.
  Device per core (all screening, fp8/fp16):
    - fp8 DoubleRow matmuls: sim[b 128, m 512] f32 in PSUM, K=512 in
      2 instructions (256 contraction each).
    - PSUM pair-drains (2 banks / op) on ACT -> fp16 slabs for most
      (g, b) blocks; DVE direct max-folds from PSUM for the rest.
    - DVE quad tensor_tensor-max folds (fp16, 2x mode) collapse the 16
      m-groups of each b-chunk to one [128, 512] array = max over
      groups at each in-group position.
    - DVE Max/MaxIndex top-8 per b row -> 8 positions per (core, b).
  Host post: candidates = {core*8192 + g*512 + pos} for all 16 g
  (position multiplicity), fp32 cosine re-rank of 1024 candidates/row,
  exact fp64 re-rank of the top 16, smallest-index tie-break, gather.

Screening margin: fp8 dot noise sigma ~9 units (of 4096-scaled sims),
gap between the global max and the 8th-best folded position is ~15
sigma, and the true argmax position is by construction the top-1
folded value of its core, so top-8 position selection cannot lose it
short of astronomically unlikely noise.
"""

import numpy as np
import ml_dtypes

import concourse.bacc as bacc
import concourse.tile as tile
from concourse import mybir
from concourse.bass_utils import run_bass_kernel_spmd

B, D, M_TOT = 512, 512, 65536
C = 8                    # cores
M = M_TOT // C           # 8192 rows per core
P = 128
TB = B // P              # 4 b-chunks
NG = M // 512            # 16 m-groups of 512
QSCALE = 64.0            # pre-quantization scale (exact power of 2)

F32 = mybir.dt.float32
FP16 = mybir.dt.float16
FP8 = mybir.dt.float8e4
U32 = mybir.dt.uint32
DR = mybir.MatmulPerfMode.DoubleRow
MAX = mybir.AluOpType.max

# chunk 3: pairs >= ACT_PAIRS3 are direct DVE drain-folds; everything else
# is ACT pair-drained into fp16 slabs and pair-folded on DVE.
ACT_PAIRS3 = 1
SLAB_OFF = {0: 0, 1: 16, 2: 32, 3: 48}

_NC_CACHE = {}


def build_nc():
    if "nc" in _NC_CACHE:
        return _NC_CACHE["nc"]
    from contextlib import ExitStack

    nc = bacc.Bacc("TRN2", target_bir_lowering=False, debug=False)
    ctx_d = nc.dram_tensor("ctxT8", [P, 4, B], FP8, kind="ExternalInput")
    mem_d = nc.dram_tensor("memT8", [P, 4, M], FP8, kind="ExternalInput")
    rb_d = nc.dram_tensor("rb", [P, TB, 512], FP16, kind="ExternalOutput")

    with tile.TileContext(nc) as tc, ExitStack() as ex:
        big = ex.enter_context(tc.tile_pool(name="big", bufs=1))
        ps = ex.enter_context(tc.tile_pool(name="ps", bufs=1, space="PSUM"))

        ctx8 = big.tile([P, 4, B], FP8)
        mem8 = big.tile([P, 4, M], FP8)
        simb = big.tile([P, 64, 512], FP16)     # ACT-drained slabs
        run2 = big.tile([P, 2, 512], F32)       # chunk-3 direct fold
        rdm = big.tile([P, 512], FP16)
        runb = big.tile([P, TB, 512], FP16)     # single running max per chunk

        acc = [ps.tile([P, 2, 512], F32, tag=f"acc{b}", name=f"acc{b}")
               for b in range(TB)]

        for b in range(TB):
            bs = slice(b * P, (b + 1) * P)
            nc.sync.dma_start(ctx8[:, :, bs], ctx_d[:, :, bs])
        for k in range(NG):
            nc.gpsimd.dma_start(mem8[:, :, k * 512:(k + 1) * 512],
                                mem_d[:, :, k * 512:(k + 1) * 512])


        def fold_slab(b, s):
            if s == 0:
                nc.vector.tensor_copy(runb[:, b, :], simb[:, SLAB_OFF[b], :])
            else:
                nc.vector.tensor_tensor(runb[:, b, :],
                                        simb[:, SLAB_OFF[b] + s, :],
                                        runb[:, b, :], MAX)

        TAIL_ORD = (0, 1, 2, 3)
        for g in range(NG):
            sl = g % 2
            for b in range(TB):
                a = acc[b][:, sl, :]
                ms = slice(g * 512, (g + 1) * 512)
                bs = slice(b * P, (b + 1) * P)
                nc.tensor.matmul(a, ctx8[:, 0:2, bs], mem8[:, 0:2, ms],
                                 start=True, stop=False, perf_mode=DR)
                nc.tensor.matmul(a, ctx8[:, 2:4, bs], mem8[:, 2:4, ms],
                                 start=False, stop=True, perf_mode=DR)
            if g <= 1:
                # lead-in: ACT single drains for c0/c1 (earliest ACT start),
                # DVE drains c2/c3 straight from PSUM in parallel
                for b in range(2):
                    nc.scalar.copy(simb[:, SLAB_OFF[b] + g, :],
                                   acc[b][:, sl, :])
                    fold_slab(b, g)
                for b in (2, 3):
                    if g == 0:
                        nc.vector.tensor_copy(runb[:, b, :],
                                              acc[b][:, sl, :])
                    else:
                        nc.vector.tensor_tensor(runb[:, b, :],
                                                acc[b][:, sl, :],
                                                runb[:, b, :], MAX)
                continue
            if g >= 14:
                # tail: ACT single drains + short DVE folds for all chunks,
                # so the final DVE chain depends only on ACT's last drains
                for b in TAIL_ORD:
                    nc.scalar.copy(simb[:, SLAB_OFF[b] + g, :],
                                   acc[b][:, sl, :])
                    fold_slab(b, g)
                if g == 15:
                    for b in TAIL_ORD:
                        nc.sync.dma_start(rb_d[:, b, :], runb[:, b, :])
                continue
            if sl == 1:
                pair = g // 2
                # chunk 3: direct DVE drain-fold for later pairs
                if pair >= ACT_PAIRS3:
                    if pair == ACT_PAIRS3:
                        nc.vector.tensor_copy(run2[:], acc[3][:])
                    else:
                        nc.vector.tensor_tensor(run2[:], acc[3][:],
                                                run2[:], MAX)
                    if pair == 6:
                        # pre-merge c3's two fold lanes off the tail path
                        nc.vector.tensor_tensor(rdm[:], run2[:, 0, :],
                                                run2[:, 1, :], MAX)
                        nc.vector.tensor_tensor(runb[:, 3, :], rdm[:],
                                                runb[:, 3, :], MAX)
                for b in range(TB):
                    if b == 3 and pair >= ACT_PAIRS3:
                        continue
                    s = SLAB_OFF[b] + 2 * pair
                    nc.scalar.copy(simb[:, s:s + 2, :], acc[b][:])
                    fold_slab(b, 2 * pair)
                    fold_slab(b, 2 * pair + 1)

    nc.compile()
    _NC_CACHE["nc"] = nc
    return nc


def _host_prep(context, memory):
    ctx = np.ascontiguousarray(context, dtype=np.float32)
    mem = np.ascontiguousarray(memory, dtype=np.float32)
    mem_n2 = np.maximum((mem * mem).sum(1, keepdims=True), 1e-12)
    mem_n = mem / np.sqrt(mem_n2)
    ctx_n2 = np.maximum((ctx * ctx).sum(1, keepdims=True), 1e-12)
    ctx_n = ctx / np.sqrt(ctx_n2)

    ctx8 = (ctx_n * QSCALE).astype(ml_dtypes.float8_e4m3)
    mem8 = (mem_n * QSCALE).astype(ml_dtypes.float8_e4m3)

    ctxT8 = np.ascontiguousarray(
        ctx8.T.reshape(4, P, B).transpose(1, 0, 2))
    mem_shards = []
    for c in range(C):
        q = mem8[c * M:(c + 1) * M]
        mem_shards.append(np.ascontiguousarray(
            q.T.reshape(4, P, M).transpose(1, 0, 2)))
    return ctx_n, mem_n, ctxT8, mem_shards


def run_device(context, memory, trace=False):
    nc = build_nc()
    _, _, ctxT8, mem_shards = _host_prep(context, memory)
    in_maps = [{"ctxT8": ctxT8, "memT8": mem_shards[c]} for c in range(C)]
    return run_bass_kernel_spmd(nc, in_maps, list(range(C)), trace=trace)


def kernel(context: np.ndarray, memory: np.ndarray) -> np.ndarray:
    nc = build_nc()
    ctx_n, mem_n, ctxT8, mem_shards = _host_prep(context, memory)
    in_maps = [{"ctxT8": ctxT8, "memT8": mem_shards[c]} for c in range(C)]
    res = run_bass_kernel_spmd(nc, in_maps, list(range(C)))

    # folded max arrays [C, P, TB, 512] -> [b, c, 512]; host top-8 positions
    rb = np.stack([res.results[c]["rb"] for c in range(C)], axis=0)
    rb_b = rb.transpose(2, 1, 0, 3).reshape(B, C, 512).astype(np.float32)
    pos_b = np.argpartition(-rb_b, 7, axis=2)[:, :, :8].astype(np.int64)
    g = np.arange(NG, dtype=np.int64)
    cand = (np.arange(C, dtype=np.int64)[None, :, None, None] * M
            + g[None, None, None, :] * 512
            + pos_b[:, :, :, None]).reshape(B, C * 8 * NG)

    # fp32 cosine prefilter over the 1024 candidates per row
    KTOP = 16
    best16 = np.empty((B, KTOP), dtype=np.int64)
    for b0 in range(0, B, 64):
        b1 = b0 + 64
        rows = mem_n[cand[b0:b1]]                      # [64, K, D] f32
        sc = np.einsum("bd,bkd->bk", ctx_n[b0:b1], rows)
        part = np.argpartition(-sc, KTOP - 1, axis=1)[:, :KTOP]
        best16[b0:b1] = np.take_along_axis(cand[b0:b1], part, axis=1)

    # exact fp64 re-rank of the survivors, smallest-index tie-break
    ctx64 = context.astype(np.float64)
    mem64 = memory.astype(np.float64)
    ctxn64 = ctx64 / np.sqrt(np.maximum((ctx64 * ctx64).sum(1, keepdims=True),
                                        1e-12))
    mnorm = np.sqrt(np.maximum((mem64 * mem64).sum(1), 1e-12))
    rows64 = mem64[best16]                             # [B, 16, D]
    cos = np.einsum("bd,bkd->bk", ctxn64, rows64) / mnorm[best16]
    best = np.empty(B, dtype=np.int64)
    for b in range(B):
        cb, vb = best16[b], cos[b]
        mx = vb.max()
        best[b] = cb[vb >= mx].min()
    return memory[best][None, :, :].astype(np.float32)
